# revision 15
# baseline (speedup 1.0000x reference)
"""Cost-volume kernel v2 for Trainium2 (8 NeuronCores, data-parallel over B*H).

cost[b,h,w,d] = mean_c left[b,h,w,c] * right[b,h,w-(d+1),c], 0 where w-d-1 < 0
Shapes: B=4, H=256, W=512, C=64, D=64 (f32). 1024 independent (b,h) rows,
128 per core.

v2 design (vs v1: bf16-in-f32 transpose-DMA loads + DRAM scratch round trip):
  - Host packs inputs as REAL bf16 in matmul-ready layout [128, pairs*W]:
    partition p = 64*(row parity) + c, free = pair*512 + w. Left pre-scaled
    by 1/C. Input DMA halves to 16.8 MB/core at full (non-transpose) rate.
  - TensorE: per (pair, 128-w block): 4 banded matmuls [K=64, M=64, N=127]
    on PE quadrants via tile_position=(64j, 64s) — j = row parity (SBUF
    partition half), s = w half (psum partition half). Window w' in
    [ws-64, ws+63); block i=0, s=0 head (w'<0) is memset to 0 with a
    shortened matmul.
  - DVE/ACT evict psum [128, 508] (two blocks) -> bf16 rect in SBUF; rect
    is DMAed contiguously straight to the DRAM output (no scratch, no
    DRAM->DRAM shear). Device output IS the sheared rect:
      out[64s+q, (4*pr+i)*254 + 127j + q + d'] = cost[2pr+j, 128i+64s+q, 63-d']
  - Host extracts the band with one as_strided view + cast per core.
Per-core traffic: in 16.8 MB + out rect 16.6 MB = 33.4 MB ~ 93 us at
358 GB/s; TensorE ~55 us warm => memory-bound, target ~100-110 us.
"""

import numpy as np

N_CORES = 8
B_FULL, H_FULL, W, C = 4, 256, 512, 64
D = 64
ROWS = B_FULL * H_FULL           # 1024 independent rows
ROWS_PER_CORE = ROWS // N_CORES  # 128
PAIRS = ROWS_PER_CORE // 2       # 64 row pairs (2 rows share 128 partitions)
NBLK = W // 128                  # 4 w-blocks per row
BCOL = 2 * 127                   # rect cols per 128-w block (two rows)
FL = PAIRS * W                   # input free length per partition


def build_nc_v2(pg=4, lt_bufs=3, rect_bufs=3, ps_bufs=6, repeat=1,
                st_eng="scalar", rt_eng="sync", ev_vvs=False, store_split=1,
                skip_compute=False, skip_in=False, skip_store=False,
                mm_half=False, zp=False):
    import concourse.mybir as mybir
    import concourse.tile as tile
    from concourse import bacc

    nc = bacc.Bacc()
    left = nc.declare_dram_parameter("left", [128, FL], mybir.dt.bfloat16,
                                     isOutput=False)
    right = nc.declare_dram_parameter("right", [128, FL], mybir.dt.bfloat16,
                                      isOutput=False)
    ng = PAIRS // pg             # groups per core
    gcols = pg * NBLK * BCOL     # rect cols per group
    out = nc.declare_dram_parameter("out", [128, ng * gcols],
                                    mybir.dt.bfloat16, isOutput=True)

    with tile.TileContext(nc) as tc:
        with (
            tc.tile_pool(name="lt", bufs=lt_bufs) as lt_pool,
            tc.tile_pool(name="rt", bufs=lt_bufs) as rt_pool,
            tc.tile_pool(name="rect", bufs=rect_bufs) as rect_pool,
            tc.tile_pool(name="ps", bufs=ps_bufs, space="PSUM") as ps_pool,
        ):
          WR = W + 64 if zp else W     # Rt pair pitch (zp: 64 zero cols first)
          for _rep in range(repeat):
            for g in range(ng):
                f0 = g * pg * W
                Lt = lt_pool.tile([128, pg * W], mybir.dt.bfloat16, tag="lt")
                Rt = rt_pool.tile([128, pg * WR], mybir.dt.bfloat16, tag="rt")
                if not skip_in:
                    nc.sync.dma_start(Lt[:, :], left[:, f0:f0 + pg * W])
                    if zp:
                        # 64 zero head cols per pair segment (gpsimd, idle
                        # otherwise); DMA fills only the data region
                        for pr in range(pg):
                            nc.gpsimd.memset(
                                Rt[:, pr * WR:pr * WR + 64], 0.0)
                        for pr in range(pg):
                            getattr(nc, rt_eng).dma_start(
                                Rt[:, pr * WR + 64:(pr + 1) * WR],
                                right[:, f0 + pr * W:f0 + (pr + 1) * W])
                    else:
                        getattr(nc, rt_eng).dma_start(Rt[:, :],
                                                      right[:, f0:f0 + pg * W])

                Brect = rect_pool.tile([128, gcols], mybir.dt.bfloat16,
                                       tag="rect")
                if skip_in:
                    # keep tiles "written" so Tile release checks pass
                    nc.gpsimd.memset(Lt[:, 0:8], 0.0)
                    nc.gpsimd.memset(Rt[:, 0:8], 0.0)
                if skip_compute:
                    nc.gpsimd.memset(Brect[:, 0:8], 0.0)
                nev = 0
                for pr in range(pg if not skip_compute else 0):
                    rb = pr * W
                    rbR = pr * WR + 64 if zp else rb
                    # PE row-tiles (j=0/j=1 quadrant rows) must NOT write
                    # the same PSUM bank concurrently -> one bank per j,
                    # each collecting all 4 blocks of the pair.
                    PA = ps_pool.tile([128, 512], mybir.dt.float32, tag="ps")
                    PB = ps_pool.tile([128, 512], mybir.dt.float32, tag="ps")
                    PJ = [PA, PB]
                    for j in range(2):       # row of the pair (PE row tile)
                        P = PJ[j]
                        p0 = 64 * j
                        for i in range(NBLK):
                            cc = i * 127
                            for s in range(2):   # w half (PE col tile)
                                ws = i * 128 + 64 * s
                                pp = 64 * s
                                lhsT = Lt[p0:p0 + 64, rb + ws:rb + ws + 64]
                                if zp or not (i == 0 and s == 0):
                                    nw = 63 if mm_half else 127
                                    nc.tensor.matmul(
                                        P[pp:pp + 64, cc:cc + nw],
                                        lhsT,
                                        Rt[p0:p0 + 64,
                                           rbR + ws - 64:rbR + ws - 64 + nw],
                                        start=True, stop=True,
                                        tile_position=(p0, pp))
                                else:
                                    # w' < 0 head: zero, compute tail
                                    # (gpsimd/ACT cannot memset PSUM)
                                    nc.vector.memset(
                                        P[pp:pp + 64, cc:cc + 64], 0.0)
                                    nc.tensor.matmul(
                                        P[pp:pp + 64, cc + 64:cc + 127],
                                        lhsT,
                                        Rt[p0:p0 + 64, rb:rb + 63],
                                        start=True, stop=True,
                                        tile_position=(p0, pp))
                        col0 = (pr * 2 + j) * NBLK * 127
                        ev_dst = Brect[:, col0:col0 + NBLK * 127]
                        use_vec = (nev % 3 != 2) if ev_vvs else (nev % 2 == 0)
                        if use_vec:
                            nc.vector.tensor_copy(ev_dst,
                                                  P[:, 0:NBLK * 127])
                        else:
                            nc.scalar.copy(ev_dst, P[:, 0:NBLK * 127])
                        nev += 1

                if not skip_store:
                    cs = gcols // store_split
                    for sc in range(store_split):
                        getattr(nc, st_eng).dma_start(
                            out[:, g * gcols + sc * cs:
                                g * gcols + (sc + 1) * cs],
                            Brect[:, sc * cs:(sc + 1) * cs])

    nc.compile()
    return nc


def _pack(x, scale):
    """[128 rows, W, C] f32 -> [128, PAIRS*W] bf16, p = 64*parity + c."""
    import ml_dtypes
    x = np.asarray(x, dtype=np.float32)
    if scale != 1.0:
        x = x * scale
    x = x.reshape(PAIRS, 2, W, C).transpose(1, 3, 0, 2)   # [j, c, pair, w]
    return np.ascontiguousarray(x.reshape(128, FL)).astype(ml_dtypes.bfloat16)


_NC_CACHE = {}


def _unshear(o, pg=4):
    """Device rect [128, ng*gcols] bf16 -> cost [128 rows, W, D] view-copy."""
    gcols = pg * NBLK * BCOL
    o = np.asarray(o).reshape(128, (PAIRS // pg) * gcols)
    es = o.strides[-1]           # element stride in bytes (2)
    # V[s, q, pr, j, i, d'] = o[64s+q, ((pr*2+j)*NBLK+i)*127 + q + d']
    V = np.lib.stride_tricks.as_strided(
        o,
        shape=(2, 64, PAIRS, 2, NBLK, 64),
        strides=(64 * o.shape[1] * es, (o.shape[1] + 1) * es,
                 2 * NBLK * 127 * es, NBLK * 127 * es, 127 * es, es))
    # -> [pr, j, i, s, q, d'] = [row, w, d'] ; flip d' -> d
    return V.transpose(2, 3, 4, 0, 1, 5).reshape(ROWS_PER_CORE, W, D)[:, :, ::-1]


def kernel(left_feature, right_feature):
    from concourse.bass_utils import run_bass_kernel_spmd

    lf = np.asarray(left_feature, np.float32).reshape(ROWS, W, C)
    rf = np.asarray(right_feature, np.float32).reshape(ROWS, W, C)

    if "nc" not in _NC_CACHE:
        _NC_CACHE["nc"] = build_nc_v2()
    nc = _NC_CACHE["nc"]

    in_maps = []
    for k in range(N_CORES):
        sl = slice(k * ROWS_PER_CORE, (k + 1) * ROWS_PER_CORE)
        in_maps.append({
            "left": _pack(lf[sl], 1.0 / C),
            "right": _pack(rf[sl], 1.0),
        })

    res = run_bass_kernel_spmd(nc, in_maps, core_ids=list(range(N_CORES)))

    out = np.empty((ROWS, W, D), dtype=np.float32)
    for k in range(N_CORES):
        out[k * ROWS_PER_CORE:(k + 1) * ROWS_PER_CORE] = _unshear(
            res.results[k]["out"])
    return out.reshape(B_FULL, H_FULL, W, D)
